# revision 2
# baseline (speedup 1.0000x reference)
"""Trainium2 Bass kernel v3 for nn_Gate_Net (sigmoid gate cumprod).

v1 pipeline structure (full-width stripes, proven overlap) with:
  - host-gathered docs input (sharding: each core receives its documents'
    score values) instead of 258 on-device indirect DMAs (~330us saved)
  - ACT sigmoid writes fp16 directly; DVE scan runs fp16 in/out;
    output stripes DMA'd as fp16 (no separate downcast pass)

Per block of 128 docs (2 blocks/core):
  - PE-transpose docs -> F/B [t,d]; split fp32 -> bf16 hi/mid
  - arg[d,(j,k)] = sum_p F[p,d]*W[p,(j,k)] as 2 accumulating bf16 matmuls
      W_fwd[p,(j,k)] = [p==j-k] - [p==j],  W_bwd[p,(j,k)] = [p==j+k+1] - [p==j]
  - ACT: gate16 = sigmoid(100*psum + 5)  (fp16 out)
  - DVE tensor_tensor_scan(op0=max mask reset, op1=mult) fp16
  - stripe DMA out fp16
"""

import sys

sys.path.insert(0, "/opt/trn_rl_repo")

import numpy as np
import ml_dtypes

import concourse.bacc as bacc
import concourse.bass as bass
import concourse.tile as tile
from concourse import mybir
from concourse.bass_utils import run_bass_kernel_spmd

N_CORES = 8
POOL = 300000
N_DOCS = 2048
DOC_LEN = 129
L = DOC_LEN - 1          # 128
K = L - 1                # 127
JK = L * K               # 16256
DOCS_PER_CORE = N_DOCS // N_CORES  # 256
BLOCKS = DOCS_PER_CORE // 128      # 2
STRIPE = 2048
MM_WIN = 512

_BF16 = ml_dtypes.bfloat16


def _build_consts():
    j = np.arange(L)[:, None]
    k = np.arange(K)[None, :]
    p = np.arange(128)[:, None, None]
    w_fwd = ((j[None] - k[None]) == p).astype(np.float32) - (
        (j[None] == p) & np.ones_like(k[None], bool)
    ).astype(np.float32)
    w_bwd = ((j[None] + k[None] + 1) == p).astype(np.float32) - (
        (j[None] == p) & np.ones_like(k[None], bool)
    ).astype(np.float32)
    w_fwd = w_fwd.reshape(128, JK).astype(_BF16)
    w_bwd = w_bwd.reshape(128, JK).astype(_BF16)
    ident = np.eye(128, dtype=np.float32)
    smask = np.zeros((1, STRIPE + 128), np.float32)
    smask[:, ::K] = 1.0
    return w_fwd, w_bwd, ident, smask


def build_program():
    nc = bacc.Bacc("TRN2", target_bir_lowering=False, debug=False,
                   num_swdge_queues=2)
    f32 = mybir.dt.float32
    bf16 = mybir.dt.bfloat16
    fp16 = mybir.dt.float16

    docs_d = nc.dram_tensor("docs", [DOCS_PER_CORE, DOC_LEN], f32,
                            kind="ExternalInput")
    wf_d = nc.dram_tensor("w_fwd", [128, JK], bf16, kind="ExternalInput")
    wb_d = nc.dram_tensor("w_bwd", [128, JK], bf16, kind="ExternalInput")
    id_d = nc.dram_tensor("ident", [128, 128], f32, kind="ExternalInput")
    sm_d = nc.dram_tensor("smask", [1, STRIPE + 128], f32,
                          kind="ExternalInput")
    out_d = nc.dram_tensor("out", [2, DOCS_PER_CORE, JK], fp16,
                           kind="ExternalOutput")

    with tile.TileContext(nc) as tc:
        with (
            tc.tile_pool(name="consts", bufs=1) as consts,
            tc.tile_pool(name="prep", bufs=6) as prep,
            tc.tile_pool(name="gates", bufs=3) as gates,
            tc.tile_pool(name="outs", bufs=4) as outs,
            tc.tile_pool(name="psum", bufs=2, space="PSUM") as psum,
        ):
            ident = consts.tile([128, 128], f32)
            nc.scalar.dma_start(ident[:], id_d[:])

            w_sb = {}
            for dname, dram in (("f", wf_d), ("b", wb_d)):
                wt = consts.tile([128, JK], bf16, tag=f"w_{dname}")
                for c0 in range(0, JK, 4064):
                    cl = min(4064, JK - c0)
                    nc.scalar.dma_start(wt[:, c0:c0 + cl], dram[:, c0:c0 + cl])
                w_sb[dname] = wt

            mask = consts.tile([128, STRIPE + 128], f32)
            nc.sync.dma_start(mask[:],
                              sm_d[:].broadcast_to([128, STRIPE + 128]))

            bias5 = consts.tile([128, 1], f32)
            nc.gpsimd.memset(bias5[:], 5.0)

            for blk in range(BLOCKS):
                docs = consts.tile([128, DOC_LEN], f32, tag=f"docs{blk}")
                nc.sync.dma_start(docs[:],
                                  docs_d[blk * 128:(blk + 1) * 128, :])
                splits = {}
                for dname, off in (("f", 0), ("b", 1)):
                    ps = psum.tile([128, STRIPE], f32, tag="mm")
                    tps = ps[:, 0:128]
                    nc.tensor.transpose(tps, docs[:, off:off + 128], ident[:])
                    hi = consts.tile([128, 128], bf16, tag=f"hi{blk}{dname}")
                    nc.scalar.copy(hi[:], tps)
                    hi32 = prep.tile([128, 128], f32, tag="t32")
                    nc.vector.tensor_copy(hi32[:], hi[:])
                    t1 = prep.tile([128, 128], f32, tag="t32")
                    nc.vector.tensor_sub(t1[:], tps, hi32[:])
                    mid = consts.tile([128, 128], bf16, tag=f"mid{blk}{dname}")
                    nc.vector.tensor_copy(mid[:], t1[:])
                    splits[(blk, dname)] = [hi, mid]

                for di, dname in enumerate(("f", "b")):
                    wt = w_sb[dname]
                    sp = splits[(blk, dname)]
                    prev_out = None
                    prev_len = 0
                    stripes = []
                    _c = 0
                    while _c < JK:
                        _l = min(STRIPE, JK - _c)
                        stripes.append((_c, _l))
                        _c += _l
                    for s, (c0, ln) in enumerate(stripes):
                        ps = psum.tile([128, STRIPE], f32, tag="mm")
                        for si in range(len(sp)):
                            for w0 in range(0, ln, MM_WIN):
                                wl = min(MM_WIN, ln - w0)
                                nc.tensor.matmul(
                                    ps[:, w0:w0 + wl],
                                    sp[si][:],
                                    wt[:, c0 + w0:c0 + w0 + wl],
                                    start=(si == 0),
                                    stop=(si == len(sp) - 1),
                                )
                        gate = gates.tile([128, STRIPE], f32)
                        nc.scalar.activation(
                            gate[:, :ln], ps[:, :ln],
                            mybir.ActivationFunctionType.Sigmoid,
                            bias=bias5[:], scale=100.0,
                        )
                        ot = outs.tile([128, STRIPE], fp16)
                        q = c0 % K
                        init = 0.0 if s == 0 else prev_out[:,
                                                           prev_len - 1:prev_len]
                        nc.vector.tensor_tensor_scan(
                            out=ot[:, :ln],
                            data0=mask[:, q:q + ln],
                            data1=gate[:, :ln],
                            initial=init,
                            op0=mybir.AluOpType.max,
                            op1=mybir.AluOpType.mult,
                        )
                        eng = nc.sync if (s % 2 == 0) else nc.gpsimd
                        eng.dma_start(
                            out_d[di, blk * 128:(blk + 1) * 128, c0:c0 + ln],
                            ot[:, :ln],
                        )
                        prev_out, prev_len = ot, ln

    nc.compile()
    return nc


_NC = None


def _get_nc():
    global _NC
    if _NC is None:
        _NC = build_program()
    return _NC


def kernel(score, score_idx):
    score = np.ascontiguousarray(np.asarray(score, dtype=np.float32))
    idx = np.ascontiguousarray(np.asarray(score_idx).astype(np.int64))
    assert score.shape == (POOL,) and idx.shape == (N_DOCS, DOC_LEN)

    # host-side shard prep: each core receives its documents' score values
    docs_full = np.ascontiguousarray(score[idx])

    w_fwd, w_bwd, ident, smask = _build_consts()
    nc = _get_nc()

    in_maps = []
    for c in range(N_CORES):
        in_maps.append({
            "docs": docs_full[c * DOCS_PER_CORE:(c + 1) * DOCS_PER_CORE],
            "w_fwd": w_fwd,
            "w_bwd": w_bwd,
            "ident": ident,
            "smask": smask,
        })
    res = run_bass_kernel_spmd(nc, in_maps, core_ids=list(range(N_CORES)))
    shards = [np.asarray(r["out"]).astype(np.float32).reshape(
        2, DOCS_PER_CORE, L, K) for r in res.results]
    return np.concatenate(shards, axis=1)


if __name__ == "__main__":
    rng = np.random.default_rng(0)
    score = rng.standard_normal(POOL).astype(np.float32)
    idx = rng.integers(0, POOL, size=(N_DOCS, DOC_LEN)).astype(np.int64)
    out = kernel(score, idx)
    print(out.shape, out.dtype, float(out[0, 0, :4, :4].sum()))


# revision 3
# speedup vs baseline: 1.0216x; 1.0216x over previous
"""Trainium2 Bass kernel v3 for nn_Gate_Net (sigmoid gate cumprod).

v1 pipeline structure (full-width stripes, proven overlap) with:
  - host-gathered docs input (sharding: each core receives its documents'
    score values) instead of 258 on-device indirect DMAs (~330us saved)
  - ACT sigmoid writes fp16 directly; DVE scan runs fp16 in/out;
    output stripes DMA'd as fp16 (no separate downcast pass)

Per block of 128 docs (2 blocks/core):
  - PE-transpose docs -> F/B [t,d]; split fp32 -> bf16 hi/mid
  - arg[d,(j,k)] = sum_p F[p,d]*W[p,(j,k)] as 2 accumulating bf16 matmuls
      W_fwd[p,(j,k)] = [p==j-k] - [p==j],  W_bwd[p,(j,k)] = [p==j+k+1] - [p==j]
  - ACT: gate16 = sigmoid(100*psum + 5)  (fp16 out)
  - DVE tensor_tensor_scan(op0=max mask reset, op1=mult) fp16
  - stripe DMA out fp16
"""

import sys

sys.path.insert(0, "/opt/trn_rl_repo")

import numpy as np
import ml_dtypes

import concourse.bacc as bacc
import concourse.bass as bass
import concourse.tile as tile
from concourse import mybir
from concourse.bass_utils import run_bass_kernel_spmd

N_CORES = 8
POOL = 300000
N_DOCS = 2048
DOC_LEN = 129
L = DOC_LEN - 1          # 128
K = L - 1                # 127
JK = L * K               # 16256
DOCS_PER_CORE = N_DOCS // N_CORES  # 256
BLOCKS = DOCS_PER_CORE // 128      # 2
STRIPE = 2048
MM_WIN = 512

_BF16 = ml_dtypes.bfloat16
_FP8 = ml_dtypes.float8_e4m3


def _build_consts():
    j = np.arange(L)[:, None]
    k = np.arange(K)[None, :]
    p = np.arange(128)[:, None, None]
    w_fwd = ((j[None] - k[None]) == p).astype(np.float32) - (
        (j[None] == p) & np.ones_like(k[None], bool)
    ).astype(np.float32)
    w_bwd = ((j[None] + k[None] + 1) == p).astype(np.float32) - (
        (j[None] == p) & np.ones_like(k[None], bool)
    ).astype(np.float32)
    w_fwd = w_fwd.reshape(128, JK).astype(_FP8)
    w_bwd = w_bwd.reshape(128, JK).astype(_FP8)
    ident = np.eye(128, dtype=np.float32)
    smask = np.zeros((1, STRIPE + 128), np.float32)
    smask[:, ::K] = 1.0
    return w_fwd, w_bwd, ident, smask


def build_program():
    nc = bacc.Bacc("TRN2", target_bir_lowering=False, debug=False,
                   num_swdge_queues=2)
    f32 = mybir.dt.float32
    bf16 = mybir.dt.bfloat16
    fp16 = mybir.dt.float16
    fp8 = mybir.dt.float8e4

    docs_d = nc.dram_tensor("docs", [DOCS_PER_CORE, DOC_LEN], f32,
                            kind="ExternalInput")
    wf_d = nc.dram_tensor("w_fwd", [128, JK], fp8, kind="ExternalInput")
    wb_d = nc.dram_tensor("w_bwd", [128, JK], fp8, kind="ExternalInput")
    id_d = nc.dram_tensor("ident", [128, 128], f32, kind="ExternalInput")
    sm_d = nc.dram_tensor("smask", [1, STRIPE + 128], f32,
                          kind="ExternalInput")
    out_d = nc.dram_tensor("out", [2, DOCS_PER_CORE, JK], fp16,
                           kind="ExternalOutput")

    with tile.TileContext(nc) as tc:
        with (
            tc.tile_pool(name="consts", bufs=1) as consts,
            tc.tile_pool(name="prep", bufs=6) as prep,
            tc.tile_pool(name="gates", bufs=3) as gates,
            tc.tile_pool(name="outs", bufs=4) as outs,
            tc.tile_pool(name="psum", bufs=2, space="PSUM") as psum,
        ):
            ident = consts.tile([128, 128], f32)
            nc.scalar.dma_start(ident[:], id_d[:])

            w_sb = {}
            for dname, dram in (("f", wf_d), ("b", wb_d)):
                wt = consts.tile([128, JK], fp8, tag=f"w_{dname}")
                for c0 in range(0, JK, 4064):
                    cl = min(4064, JK - c0)
                    nc.scalar.dma_start(wt[:, c0:c0 + cl], dram[:, c0:c0 + cl])
                w_sb[dname] = wt

            mask = consts.tile([128, STRIPE + 128], f32)
            nc.sync.dma_start(mask[:],
                              sm_d[:].broadcast_to([128, STRIPE + 128]))

            bias5 = consts.tile([128, 1], f32)
            nc.gpsimd.memset(bias5[:], 5.0)

            splits = {}
            for blk in range(BLOCKS):
                docs = consts.tile([128, DOC_LEN], f32, tag=f"docs{blk}")
                nc.sync.dma_start(docs[:],
                                  docs_d[blk * 128:(blk + 1) * 128, :])
                for dname, off in (("f", 0), ("b", 1)):
                    ps = psum.tile([128, STRIPE], f32, tag="mm")
                    tps = ps[:, 0:128]
                    nc.tensor.transpose(tps, docs[:, off:off + 128], ident[:])
                    hi = consts.tile([128, 128], bf16, tag=f"hi{blk}{dname}")
                    nc.scalar.copy(hi[:], tps)
                    hi32 = prep.tile([128, 128], f32, tag="t32")
                    nc.vector.tensor_copy(hi32[:], hi[:])
                    t1 = prep.tile([128, 128], f32, tag="t32")
                    nc.vector.tensor_sub(t1[:], tps, hi32[:])
                    mid = consts.tile([128, 128], bf16, tag=f"mid{blk}{dname}")
                    nc.vector.tensor_copy(mid[:], t1[:])
                    splits[(blk, dname)] = [hi, mid]

            for blk in range(BLOCKS):
                for di, dname in enumerate(("f", "b")):
                    wt = w_sb[dname]
                    sp = splits[(blk, dname)]
                    prev_out = None
                    prev_len = 0
                    stripes = []
                    _c = 0
                    first = (blk == 0 and di == 0)
                    while _c < JK:
                        if first and _c == 0:
                            _l = 512
                        else:
                            _l = min(STRIPE, JK - _c)
                        stripes.append((_c, _l))
                        _c += _l
                    for s, (c0, ln) in enumerate(stripes):
                        ps = psum.tile([128, STRIPE], f32, tag="mm")
                        for si in range(len(sp)):
                            for w0 in range(0, ln, MM_WIN):
                                wl = min(MM_WIN, ln - w0)
                                nc.tensor.matmul(
                                    ps[:, w0:w0 + wl],
                                    sp[si][:],
                                    wt[:, c0 + w0:c0 + w0 + wl],
                                    start=(si == 0),
                                    stop=(si == len(sp) - 1),
                                )
                        gate = gates.tile([128, STRIPE], f32)
                        nc.scalar.activation(
                            gate[:, :ln], ps[:, :ln],
                            mybir.ActivationFunctionType.Sigmoid,
                            bias=bias5[:], scale=100.0,
                        )
                        ot = outs.tile([128, STRIPE], fp16)
                        q = c0 % K
                        init = 0.0 if s == 0 else prev_out[:,
                                                           prev_len - 1:prev_len]
                        nc.vector.tensor_tensor_scan(
                            out=ot[:, :ln],
                            data0=mask[:, q:q + ln],
                            data1=gate[:, :ln],
                            initial=init,
                            op0=mybir.AluOpType.max,
                            op1=mybir.AluOpType.mult,
                        )
                        eng = nc.sync if (s % 2 == 0 or
                                          s == len(stripes) - 1) else nc.gpsimd
                        eng.dma_start(
                            out_d[di, blk * 128:(blk + 1) * 128, c0:c0 + ln],
                            ot[:, :ln],
                        )
                        prev_out, prev_len = ot, ln

    nc.compile()
    return nc


_NC = None


def _get_nc():
    global _NC
    if _NC is None:
        _NC = build_program()
    return _NC


def kernel(score, score_idx):
    score = np.ascontiguousarray(np.asarray(score, dtype=np.float32))
    idx = np.ascontiguousarray(np.asarray(score_idx).astype(np.int64))
    assert score.shape == (POOL,) and idx.shape == (N_DOCS, DOC_LEN)

    # host-side shard prep: each core receives its documents' score values
    docs_full = np.ascontiguousarray(score[idx])

    w_fwd, w_bwd, ident, smask = _build_consts()
    nc = _get_nc()

    in_maps = []
    for c in range(N_CORES):
        in_maps.append({
            "docs": docs_full[c * DOCS_PER_CORE:(c + 1) * DOCS_PER_CORE],
            "w_fwd": w_fwd,
            "w_bwd": w_bwd,
            "ident": ident,
            "smask": smask,
        })
    res = run_bass_kernel_spmd(nc, in_maps, core_ids=list(range(N_CORES)))
    shards = [np.asarray(r["out"]).astype(np.float32).reshape(
        2, DOCS_PER_CORE, L, K) for r in res.results]
    return np.concatenate(shards, axis=1)


if __name__ == "__main__":
    rng = np.random.default_rng(0)
    score = rng.standard_normal(POOL).astype(np.float32)
    idx = rng.integers(0, POOL, size=(N_DOCS, DOC_LEN)).astype(np.int64)
    out = kernel(score, idx)
    print(out.shape, out.dtype, float(out[0, 0, :4, :4].sum()))
